# revision 3
# baseline (speedup 1.0000x reference)
"""Trainium2 Bass kernel for nn_BERTSyntaxRel (biaffine syntax-relation head).

Computation (per batch b, token t):
    appended = concat([root, x[b]])                      # (S+1, D)
    gathered = appended[head_id[b, t]]                   # (D,)
    head = relu(gathered @ Wh + bh)                      # (H,)
    tail = relu(x[b, t] @ Wt + bt)                       # (H,)
    out[b, t, r] = sum_{h,k} head[h] * K[h, r, k] * tail[k]

Sharding: data-parallel over batch, 4 batches per core on 8 cores.
Weights replicated.  No collectives needed.

Key restructure: the gather commutes with the row-wise head FF, so we
compute head_all = relu([root; x[b]] @ Wh + bh) for ALL positions first
(same FLOPs), write the (1025, H) per-batch table to DRAM, and gather
H=128-float rows instead of D=768-float x rows.
"""

import numpy as np

B, S, D, H, R = 32, 1024, 768, 128, 48
NCORES = 8
BPC = B // NCORES            # batches per core (4)
TOK = BPC * S                # tokens per core (4096)
P = 128                      # partition dim / token tile
NTILES = TOK // P            # 32 token tiles per core
TBL = S + 1                  # rows per batch gather table (1025)
DC = D // P                  # 6 contraction chunks of 128
RKCH = 12                    # biaffine free-dim chunks of 512 (R*H = 6144)
RPC = 4                      # r values per 512-chunk


def build_program():
    """Build the Bass program (shared by all 8 cores, SPMD)."""
    from contextlib import ExitStack

    import concourse.bass as bass
    import concourse.tile as tile
    from concourse import bacc, mybir
    from concourse.masks import make_identity

    f32 = mybir.dt.float32
    i32 = mybir.dt.int32
    ts = bass.ts

    nc = bacc.Bacc(
        "TRN2",
        target_bir_lowering=False,
        debug=False,
        num_devices=NCORES,
    )

    x_ap = nc.dram_tensor("x", [TOK, D], f32, kind="ExternalInput").ap()
    gidx_ap = nc.dram_tensor("gidx", [TOK, 1], i32, kind="ExternalInput").ap()
    wh_ap = nc.dram_tensor("Wh", [D, H], f32, kind="ExternalInput").ap()
    wt_ap = nc.dram_tensor("Wt", [D, H], f32, kind="ExternalInput").ap()
    bh_ap = nc.dram_tensor("bh", [1, H], f32, kind="ExternalInput").ap()
    bt_ap = nc.dram_tensor("bt", [1, H], f32, kind="ExternalInput").ap()
    rooth_ap = nc.dram_tensor("rooth", [1, H], f32, kind="ExternalInput").ap()
    kern_ap = nc.dram_tensor("kern", [H, R * H], f32, kind="ExternalInput").ap()
    out_ap = nc.dram_tensor("out", [TOK, R], f32, kind="ExternalOutput").ap()

    with tile.TileContext(nc) as tc, ExitStack() as ctx:
        # ---- constants / weights, resident for the whole kernel ----
        const = ctx.enter_context(tc.tile_pool(name="const", bufs=1))
        ident = const.tile([P, P], f32)
        make_identity(nc, ident[:])
        ones1 = const.tile([1, P], f32)
        nc.gpsimd.memset(ones1[:], 1.0)
        whc = const.tile([P, D], f32)     # Wh chunks: whc[:, c*128:...] = Wh[c*128:(c+1)*128, :]
        wtc = const.tile([P, D], f32)
        wh3 = wh_ap.rearrange("(c p) h -> c p h", p=P)
        wt3 = wt_ap.rearrange("(c p) h -> c p h", p=P)
        for c in range(DC):
            nc.sync.dma_start(out=whc[:, ts(c, P)], in_=wh3[c])
            nc.sync.dma_start(out=wtc[:, ts(c, P)], in_=wt3[c])
        bh_sb = const.tile([1, H], f32)
        bt_sb = const.tile([1, H], f32)
        rt_sb = const.tile([1, H], f32)
        nc.sync.dma_start(out=bh_sb[:], in_=bh_ap[:])
        nc.sync.dma_start(out=bt_sb[:], in_=bt_ap[:])
        nc.sync.dma_start(out=rt_sb[:], in_=rooth_ap[:])
        ksb = const.tile([H, R * H], f32)  # 24KB/partition
        nc.sync.dma_start(out=ksb[:], in_=kern_ap[:])

        # tail (tok-major) for the whole core, kept in SBUF: 16KB/partition
        tailT_all = const.tile([P, TOK], f32)

        # per-batch gather tables in DRAM: row b*TBL is the root head state
        dram = ctx.enter_context(tc.tile_pool(name="dram", bufs=1, space="DRAM"))
        head_all = dram.tile([BPC * TBL, H], f32)
        for b in range(BPC):
            nc.sync.dma_start(out=head_all[b * TBL : b * TBL + 1, :], in_=rt_sb[:1, :])

        # ---- Phase A: transposes + FFs; fills head_all (DRAM) and tailT_all ----
        with (
            tc.tile_pool(name="xa", bufs=3) as xa_pool,
            tc.tile_pool(name="xt", bufs=2) as xt_pool,
            tc.tile_pool(name="ha", bufs=3) as ha_pool,
            tc.tile_pool(name="psA", bufs=2, space="PSUM") as psA,
            tc.tile_pool(name="psF", bufs=2, space="PSUM") as psF,
        ):
            for i in range(NTILES):
                b = i // (S // P)
                xt = xa_pool.tile([P, D], f32)
                nc.sync.dma_start(out=xt[:], in_=x_ap[ts(i, P), :])
                xT_ps = psA.tile([P, D], f32)
                for c in range(DC):
                    nc.tensor.transpose(
                        out=xT_ps[:, ts(c, P)], in_=xt[:, ts(c, P)], identity=ident[:]
                    )
                xT = xt_pool.tile([P, D], f32)
                nc.scalar.copy(out=xT[:], in_=xT_ps[:])

                hd_ps = psF.tile([P, H], f32, tag="hd")
                tl_ps = psF.tile([P, H], f32, tag="tl")
                for c in range(DC):
                    nc.tensor.matmul(
                        out=hd_ps[:], lhsT=xT[:, ts(c, P)], rhs=whc[:, ts(c, P)],
                        start=(c == 0), stop=False,
                    )
                nc.tensor.matmul(
                    out=hd_ps[:], lhsT=ones1[:1, :], rhs=bh_sb[:1, :],
                    start=False, stop=True,
                )
                for c in range(DC):
                    nc.tensor.matmul(
                        out=tl_ps[:], lhsT=xT[:, ts(c, P)], rhs=wtc[:, ts(c, P)],
                        start=(c == 0), stop=False,
                    )
                nc.tensor.matmul(
                    out=tl_ps[:], lhsT=ones1[:1, :], rhs=bt_sb[:1, :],
                    start=False, stop=True,
                )
                hA = ha_pool.tile([P, H], f32)
                nc.scalar.activation(
                    out=hA[:], in_=hd_ps[:], func=mybir.ActivationFunctionType.Relu
                )
                nc.scalar.activation(
                    out=tailT_all[:, ts(i, P)], in_=tl_ps[:],
                    func=mybir.ActivationFunctionType.Relu,
                )
                row0 = b * TBL + 1 + (i % (S // P)) * P
                nc.sync.dma_start(out=head_all[row0 : row0 + P, :], in_=hA[:])

        # head_all DRAM writes must complete before the gathers below read them
        tc.strict_bb_all_engine_barrier()

        # ---- Phase B: gather + biaffine ----
        with (
            tc.tile_pool(name="gx", bufs=3) as gx_pool,
            tc.tile_pool(name="gb", bufs=3) as gb_pool,
            tc.tile_pool(name="hb", bufs=2) as hb_pool,
            tc.tile_pool(name="prod", bufs=3) as prod_pool,
            tc.tile_pool(name="ob", bufs=3) as ob_pool,
            tc.tile_pool(name="psT", bufs=2, space="PSUM") as psT,
            tc.tile_pool(name="psM", bufs=4, space="PSUM") as psM,
        ):
            for i in range(NTILES):
                gix = gx_pool.tile([P, 1], i32)
                nc.sync.dma_start(out=gix[:], in_=gidx_ap[ts(i, P), :])
                g_sb = gb_pool.tile([P, H], f32)
                nc.gpsimd.indirect_dma_start(
                    out=g_sb[:],
                    out_offset=None,
                    in_=head_all[:],
                    in_offset=bass.IndirectOffsetOnAxis(ap=gix[:, :1], axis=0),
                )
                hT_ps = psT.tile([P, H], f32)
                nc.tensor.transpose(out=hT_ps[:], in_=g_sb[:], identity=ident[:])
                head_sb = hb_pool.tile([P, H], f32)
                nc.scalar.copy(out=head_sb[:], in_=hT_ps[:])

                out_sb = ob_pool.tile([P, R], f32)
                tl3 = (
                    tailT_all[:, ts(i, P)]
                    .rearrange("p (o k) -> p o k", o=1)
                    .to_broadcast([P, RPC, H])
                )
                for j in range(RKCH):
                    tmp_ps = psM.tile([P, 512], f32)
                    nc.tensor.matmul(
                        out=tmp_ps[:], lhsT=head_sb[:], rhs=ksb[:, ts(j, 512)],
                        start=True, stop=True,
                    )
                    prod = prod_pool.tile([P, 512], f32)
                    nc.vector.tensor_tensor(
                        out=prod[:].rearrange("p (r k) -> p r k", k=H),
                        in0=tmp_ps[:].rearrange("p (r k) -> p r k", k=H),
                        in1=tl3,
                        op=mybir.AluOpType.mult,
                    )
                    nc.vector.tensor_reduce(
                        out=out_sb[:, ts(j, RPC)],
                        in_=prod[:].rearrange("p (r k) -> p r k", k=H),
                        axis=mybir.AxisListType.X,
                        op=mybir.AluOpType.add,
                    )
                nc.sync.dma_start(out=out_ap[ts(i, P), :], in_=out_sb[:])

    nc.compile()
    return nc


def prep_inputs(x, head_id, root, Wh, bh, Wt, bt, kernel):
    """Host-side prep: shard over batch, precompute gather indices & root head."""
    x = np.asarray(x, dtype=np.float32)
    head_id = np.asarray(head_id)
    root = np.asarray(root, dtype=np.float32)
    Wh = np.asarray(Wh, dtype=np.float32)
    bh = np.asarray(bh, dtype=np.float32)
    Wt = np.asarray(Wt, dtype=np.float32)
    bt = np.asarray(bt, dtype=np.float32)
    kernel = np.asarray(kernel, dtype=np.float32)

    rooth = np.maximum(root @ Wh + bh, 0.0).astype(np.float32).reshape(1, H)
    shared = {
        "Wh": Wh,
        "Wt": Wt,
        "bh": bh.reshape(1, H).astype(np.float32),
        "bt": bt.reshape(1, H).astype(np.float32),
        "rooth": rooth,
        "kern": kernel,
    }
    in_maps = []
    for c in range(NCORES):
        bs = slice(c * BPC, (c + 1) * BPC)
        hid = head_id[bs].astype(np.int64)
        boff = (np.arange(BPC, dtype=np.int64) * TBL)[:, None]
        gidx = (hid + boff).reshape(TOK, 1).astype(np.int32)
        m = dict(shared)
        m["x"] = np.ascontiguousarray(x[bs].reshape(TOK, D))
        m["gidx"] = gidx
        in_maps.append(m)
    return in_maps


_NC_CACHE = {}


def _get_program():
    if "nc" not in _NC_CACHE:
        _NC_CACHE["nc"] = build_program()
    return _NC_CACHE["nc"]


def kernel(x, head_id, root, Wh, bh, Wt, bt, kernel):
    from concourse import bass_utils

    in_maps = prep_inputs(x, head_id, root, Wh, bh, Wt, bt, kernel)
    nc = _get_program()
    res = bass_utils.run_bass_kernel_spmd(nc, in_maps, core_ids=list(range(NCORES)))
    outs = [res.results[c]["out"].reshape(BPC, S, R) for c in range(NCORES)]
    return np.concatenate(outs, axis=0)


# revision 25
# speedup vs baseline: 1.0834x; 1.0834x over previous
"""Trainium2 Bass kernel for nn_BERTSyntaxRel (biaffine syntax-relation head).

Computation (per batch b, token t):
    appended = concat([root, x[b]])                      # (S+1, D)
    gathered = appended[head_id[b, t]]                   # (D,)
    head = relu(gathered @ Wh + bh)                      # (H,)
    tail = relu(x[b, t] @ Wt + bt)                       # (H,)
    out[b, t, r] = sum_{h,k} head[h] * K[h, r, k] * tail[k]

Sharding: data-parallel over batch, 4 batches per core on 8 cores.
Weights replicated.  No collectives needed.

Key restructure: the gather commutes with the row-wise head FF, so we
compute head_all = relu([root; x[b]] @ Wh + bh) for ALL positions first
(same FLOPs), write the (1025, H) per-batch table to DRAM, and gather
H=128-float rows instead of D=768-float x rows.
"""

import numpy as np

B, S, D, H, R = 32, 1024, 768, 128, 48
NCORES = 8
BPC = B // NCORES            # batches per core (4)
TOK = BPC * S                # tokens per core (4096)
P = 128                      # partition dim / token tile
NTILES = TOK // P            # 32 token tiles per core
TBL = S + 1                  # rows per batch gather table (1025)
DC = D // P                  # 6 contraction chunks of 128
RKCH = 12                    # biaffine free-dim chunks of 512 (R*H = 6144)
RPC = 4                      # r values per 512-chunk
import os as _os

NDVE = int(_os.environ.get("K_NDVE", "6"))  # chunks via DVE tensor_tensor_reduce;
                             # the rest go ACT-copy -> gpsimd-mul -> DVE-reduce
DEPEDGE = _os.environ.get("K_DEPEDGE", "1") == "1"  # explicit gather->table-write deps


def build_program(with_bias=True):
    """Build the Bass program (shared by all 8 cores, SPMD)."""
    from contextlib import ExitStack

    import concourse.bass as bass
    import concourse.tile as tile
    from concourse import bacc, mybir
    from concourse.masks import make_identity

    f32 = mybir.dt.float32
    i32 = mybir.dt.int32
    ts = bass.ts

    nc = bacc.Bacc(
        "TRN2",
        target_bir_lowering=False,
        debug=False,
        num_devices=NCORES,
    )

    x_ap = nc.dram_tensor("x", [TOK, D], f32, kind="ExternalInput").ap()
    gidx_ap = nc.dram_tensor("gidx", [TOK, 1], i32, kind="ExternalInput").ap()
    wh_ap = nc.dram_tensor("Wh", [D, H], f32, kind="ExternalInput").ap()
    wt_ap = nc.dram_tensor("Wt", [D, H], f32, kind="ExternalInput").ap()
    bh_ap = nc.dram_tensor("bh", [1, H], f32, kind="ExternalInput").ap()
    bt_ap = nc.dram_tensor("bt", [1, H], f32, kind="ExternalInput").ap()
    rooth_ap = nc.dram_tensor("rooth", [1, H], f32, kind="ExternalInput").ap()
    kern_ap = nc.dram_tensor("kern", [H, R * H], f32, kind="ExternalInput").ap()
    out_ap = nc.dram_tensor("out", [TOK, R], f32, kind="ExternalOutput").ap()

    with tile.TileContext(nc) as tc, ExitStack() as ctx:
        # ---- constants / weights, resident for the whole kernel ----
        const = ctx.enter_context(tc.tile_pool(name="const", bufs=1))
        ident = const.tile([P, P], f32)
        make_identity(nc, ident[:])
        ones1 = const.tile([1, P], f32)
        nc.gpsimd.memset(ones1[:], 1.0)
        # combined FF weights: per d-chunk c, wht[:, c*256 : c*256+128] = Wh chunk,
        # wht[:, c*256+128 : (c+1)*256] = Wt chunk -> one N=256 matmul per chunk
        wht = const.tile([P, 2 * D], f32)
        wh3 = wh_ap.rearrange("(c p) h -> c p h", p=P)
        wt3 = wt_ap.rearrange("(c p) h -> c p h", p=P)
        for c in range(DC):
            nc.sync.dma_start(out=wht[:, ts(2 * c, P)], in_=wh3[c])
            nc.sync.dma_start(out=wht[:, ts(2 * c + 1, P)], in_=wt3[c])
        bb_sb = const.tile([1, 2 * H], f32)
        rt_sb = const.tile([1, H], f32)
        nc.sync.dma_start(out=bb_sb[:, :H], in_=bh_ap[:])
        nc.sync.dma_start(out=bb_sb[:, H:], in_=bt_ap[:])
        nc.sync.dma_start(out=rt_sb[:], in_=rooth_ap[:])
        ksb = const.tile([H, R * H], f32)  # 24KB/partition
        nc.sync.dma_start(out=ksb[:], in_=kern_ap[:])

        # tail (tok-major) for the whole core, kept in SBUF: 16KB/partition
        tailT_all = const.tile([P, TOK], f32)

        # per-batch gather tables in DRAM: row b*TBL is the root head state
        dram = ctx.enter_context(tc.tile_pool(name="dram", bufs=1, space="DRAM"))
        head_all = dram.tile([BPC * TBL, H], f32)
        # head_all writers per batch (Tile does not track DRAM deps; the
        # Phase-B gathers get explicit dep edges on these)
        tbl_writes = [[] for _ in range(BPC)]
        for b in range(BPC):
            w = nc.sync.dma_start(
                out=head_all[b * TBL : b * TBL + 1, :], in_=rt_sb[:1, :]
            )
            tbl_writes[b].append(w.ins)

        # ---- Phase A: transposes + FFs; fills head_all (DRAM) and tailT_all ----
        with (
            tc.tile_pool(name="xa", bufs=3) as xa_pool,
            tc.tile_pool(name="xt", bufs=2) as xt_pool,
            tc.tile_pool(name="ha", bufs=3) as ha_pool,
            tc.tile_pool(name="psA", bufs=2, space="PSUM") as psA,
            tc.tile_pool(name="psF", bufs=2, space="PSUM") as psF,
        ):
            for i in range(NTILES):
                b = i // (S // P)
                xt = xa_pool.tile([P, D], f32)
                nc.sync.dma_start(out=xt[:], in_=x_ap[ts(i, P), :])
                xT_ps = psA.tile([P, D], f32)
                for c in range(DC):
                    nc.tensor.transpose(
                        out=xT_ps[:, ts(c, P)], in_=xt[:, ts(c, P)], identity=ident[:]
                    )
                xT = xt_pool.tile([P, D], f32)
                nc.scalar.copy(out=xT[:], in_=xT_ps[:])

                ff_ps = psF.tile([P, 2 * H], f32)
                for c in range(DC):
                    nc.tensor.matmul(
                        out=ff_ps[:], lhsT=xT[:, ts(c, P)], rhs=wht[:, ts(c, 2 * P)],
                        start=(c == 0), stop=(c == DC - 1 and not with_bias),
                    )
                if with_bias:
                    nc.tensor.matmul(
                        out=ff_ps[:], lhsT=ones1[:1, :], rhs=bb_sb[:1, :],
                        start=False, stop=True,
                    )
                hA = ha_pool.tile([P, H], f32)
                nc.scalar.activation(
                    out=hA[:], in_=ff_ps[:, :H], func=mybir.ActivationFunctionType.Relu
                )
                nc.scalar.activation(
                    out=tailT_all[:, ts(i, P)], in_=ff_ps[:, H:],
                    func=mybir.ActivationFunctionType.Relu,
                )
                row0 = b * TBL + 1 + (i % (S // P)) * P
                w = nc.sync.dma_start(out=head_all[row0 : row0 + P, :], in_=hA[:])
                tbl_writes[b].append(w.ins)

        # head_all DRAM writes must complete before the gathers read them
        tc.strict_bb_all_engine_barrier()

        # ---- Phase B: gather + biaffine ----
        with (
            tc.tile_pool(name="gx", bufs=3) as gx_pool,
            tc.tile_pool(name="gb", bufs=3) as gb_pool,
            tc.tile_pool(name="hb", bufs=2) as hb_pool,
            tc.tile_pool(name="prod", bufs=3) as prod_pool,
            tc.tile_pool(name="ob", bufs=3) as ob_pool,
            tc.tile_pool(name="psT", bufs=2, space="PSUM") as psT,
            tc.tile_pool(name="psM", bufs=6, space="PSUM") as psM,
        ):
            for i in range(NTILES):
                b = i // (S // P)
                gix = gx_pool.tile([P, 1], i32)
                nc.sync.dma_start(out=gix[:], in_=gidx_ap[ts(i, P), :])
                g_sb = gb_pool.tile([P, H], f32)
                g = nc.gpsimd.indirect_dma_start(
                    out=g_sb[:],
                    out_offset=None,
                    in_=head_all[:],
                    in_offset=bass.IndirectOffsetOnAxis(ap=gix[:, :1], axis=0),
                )
                if DEPEDGE:
                    for w_ins in tbl_writes[b]:
                        tile.add_dep_helper(
                            g.ins, w_ins, sync=True, reason="head_all RAW"
                        )
                hT_ps = psT.tile([P, H], f32)
                nc.tensor.transpose(out=hT_ps[:], in_=g_sb[:], identity=ident[:])
                head_sb = hb_pool.tile([P, H], f32)
                nc.scalar.copy(out=head_sb[:], in_=hT_ps[:])

                out_sb = ob_pool.tile([P, R], f32)
                tlT = tailT_all[:, ts(i, P)]
                tl3 = tlT.rearrange("p (o k) -> p o k", o=1).to_broadcast([P, RPC, H])
                for j in range(RKCH):
                    tmp_ps = psM.tile([P, 512], f32)
                    nc.tensor.matmul(
                        out=tmp_ps[:], lhsT=head_sb[:], rhs=ksb[:, ts(j, 512)],
                        start=True, stop=True,
                    )
                    if NDVE < 0:
                        # v0: plain DVE tensor_tensor mul + tensor_reduce
                        prod = prod_pool.tile([P, 512], f32, tag="pr")
                        nc.vector.tensor_tensor(
                            out=prod[:].rearrange("p (r k) -> p r k", k=H),
                            in0=tmp_ps[:].rearrange("p (r k) -> p r k", k=H),
                            in1=tl3,
                            op=mybir.AluOpType.mult,
                        )
                        nc.vector.tensor_reduce(
                            out=out_sb[:, ts(j, RPC)],
                            in_=prod[:].rearrange("p (r k) -> p r k", k=H),
                            axis=mybir.AxisListType.X,
                            op=mybir.AluOpType.add,
                        )
                    elif j < NDVE:
                        # fused mul+reduce per r on DVE (reads tmp from PSUM)
                        scr = prod_pool.tile([P, 512], f32, tag="scr")
                        for q in range(RPC):
                            r = j * RPC + q
                            nc.vector.tensor_tensor_reduce(
                                out=scr[:, ts(q, H)],
                                in0=tmp_ps[:, ts(q, H)],
                                in1=tlT,
                                scale=1.0,
                                scalar=0.0,
                                op0=mybir.AluOpType.mult,
                                op1=mybir.AluOpType.add,
                                accum_out=out_sb[:, r : r + 1],
                            )
                    else:
                        # ACT evacuates PSUM, gpsimd multiplies, DVE reduces
                        cp = prod_pool.tile([P, 512], f32, tag="cp")
                        nc.scalar.copy(out=cp[:], in_=tmp_ps[:])
                        pr = prod_pool.tile([P, 512], f32, tag="pr")
                        nc.gpsimd.tensor_tensor(
                            out=pr[:].rearrange("p (r k) -> p r k", k=H),
                            in0=cp[:].rearrange("p (r k) -> p r k", k=H),
                            in1=tl3,
                            op=mybir.AluOpType.mult,
                        )
                        nc.vector.tensor_reduce(
                            out=out_sb[:, ts(j, RPC)],
                            in_=pr[:].rearrange("p (r k) -> p r k", k=H),
                            axis=mybir.AxisListType.X,
                            op=mybir.AluOpType.add,
                        )
                nc.sync.dma_start(out=out_ap[ts(i, P), :], in_=out_sb[:])

    nc.compile()
    return nc


def prep_inputs(x, head_id, root, Wh, bh, Wt, bt, kernel):
    """Host-side prep: shard over batch, precompute gather indices & root head."""
    x = np.asarray(x, dtype=np.float32)
    head_id = np.asarray(head_id)
    root = np.asarray(root, dtype=np.float32)
    Wh = np.asarray(Wh, dtype=np.float32)
    bh = np.asarray(bh, dtype=np.float32)
    Wt = np.asarray(Wt, dtype=np.float32)
    bt = np.asarray(bt, dtype=np.float32)
    kernel = np.asarray(kernel, dtype=np.float32)

    rooth = np.maximum(root @ Wh + bh, 0.0).astype(np.float32).reshape(1, H)
    shared = {
        "Wh": Wh,
        "Wt": Wt,
        "bh": bh.reshape(1, H).astype(np.float32),
        "bt": bt.reshape(1, H).astype(np.float32),
        "rooth": rooth,
        "kern": kernel,
    }
    in_maps = []
    for c in range(NCORES):
        bs = slice(c * BPC, (c + 1) * BPC)
        hid = head_id[bs].astype(np.int64)
        boff = (np.arange(BPC, dtype=np.int64) * TBL)[:, None]
        gidx = (hid + boff).reshape(TOK, 1).astype(np.int32)
        m = dict(shared)
        m["x"] = np.ascontiguousarray(x[bs].reshape(TOK, D))
        m["gidx"] = gidx
        in_maps.append(m)
    return in_maps


_NC_CACHE = {}


def _get_program(with_bias=True):
    key = ("nc", with_bias)
    if key not in _NC_CACHE:
        _NC_CACHE[key] = build_program(with_bias=with_bias)
    return _NC_CACHE[key]


def kernel(x, head_id, root, Wh, bh, Wt, bt, kernel):
    from concourse import bass_utils

    in_maps = prep_inputs(x, head_id, root, Wh, bh, Wt, bt, kernel)
    with_bias = bool(np.any(np.asarray(bh)) or np.any(np.asarray(bt)))
    nc = _get_program(with_bias=with_bias)
    res = bass_utils.run_bass_kernel_spmd(nc, in_maps, core_ids=list(range(NCORES)))
    outs = [res.results[c]["out"].reshape(BPC, S, R) for c in range(NCORES)]
    return np.concatenate(outs, axis=0)
